# revision 16
# baseline (speedup 1.0000x reference)
"""Trainium2 Bass kernel for a dense graph-transformer block (fp8 DoubleRow).

Reference computation (per batch item b, with C=256, N=H*W=1024):
    nodes = x[b].reshape(C, N).T                      # [N, C]
    q     = nodes @ proj_w.T + proj_b                 # [N, C]
    S     = (q @ q.T) / sqrt(C)                       # [N, N]  (symmetric!)
    A     = softmax(S, axis=-1)
    agg   = A @ nodes                                 # [N, C]
    h     = gelu(agg @ w1.T + b1)  (erf gelu)
    out   = h @ w2.T + b2
    y[b]  = x[b] + out.T.reshape(C, H, W)

Strategy (data-parallel over batch, 2 items per core, 8 cores):
  Everything stays in the transposed layout [C-on-partitions, N-free].
  All heavy matmuls run in fp8 DoubleRow perf mode (2 k-tiles folded per
  instruction, 0.5 cycles per output row):
    - qT   = proj_w @ X + pb      (bf16 matmul; DVE adds bias, emits e4m3)
    - S    = qT.T @ qT            (e4m3 x e4m3 DoubleRow)
    - E    = exp(S/16 - 4.5)      (ACT, emits e5m2; e5m2's wide exponent
                                   absorbs the full score range with a single
                                   global shift that cancels in softmax)
    - Z    = ones.T @ E           (e4m3 ones x e5m2 E DoubleRow; E symmetric
                                   so column sums equal row sums)
    - agg  = XT.T @ E             (e4m3 XT x e5m2 E DoubleRow), scaled by
                                   broadcast(1/Z) (gpsimd partition_broadcast)
    - mlp  = two more e4m3 DoubleRow layers; gelu fused on ACT.
  XT ships pre-transposed from the host (layout prep, zero flops), as do the
  fp8/bf16 weight casts; all big tensors ship p-major packed so each DMA is
  128 descriptors of >=2KB.  The two items are software-pipelined phase by
  phase so ACT (the bottleneck: 16 exps + 4 gelus) never starves, and ACT
  switches tables (exp<->gelu) only twice per launch.
"""

import os
import sys

import numpy as np

for _p in ("/opt/trn_rl_repo", "/root/.axon_site/_ro/trn_rl_repo"):
    if os.path.isdir(_p) and _p not in sys.path:
        sys.path.insert(0, _p)

import ml_dtypes
import concourse.bass as bass
import concourse.bacc as bacc
import concourse.mybir as mybir
from concourse import tile
from concourse.alu_op_type import AluOpType
from concourse.bass_utils import run_bass_kernel_spmd

F32 = mybir.dt.float32
BF16 = mybir.dt.bfloat16
E4 = mybir.dt.float8e4
E5 = mybir.dt.float8e5
AFT = mybir.ActivationFunctionType
DR = mybir.MatmulPerfMode.DoubleRow

NP_E4 = ml_dtypes.float8_e4m3
NP_E5 = ml_dtypes.float8_e5m2
NP_BF = ml_dtypes.bfloat16

C = 256          # channels
N = 1024         # nodes = H*W
CT = C // 128    # channel partition-tiles (2)
NT = N // 128    # node partition-tiles (8)
NF = N // 512    # node free-chunks of 512 (2)
N_CORES = 8
ITEMS = 2        # batch items per core (B=16 / 8 cores)
EXP_BIAS = -4.5  # global exp shift; cancels in softmax, keeps E in e5m2 range


def ts(i, size):
    return slice(i * size, (i + 1) * size)


def build_nc():
    nc = bacc.Bacc(None, target_bir_lowering=False)

    xbf_d = nc.dram_tensor("xbf", [ITEMS, 128, CT * N], BF16, kind="ExternalInput")
    xt8_d = nc.dram_tensor("xt8", [ITEMS, 128, NT * C], E4, kind="ExternalInput")
    pwT_d = nc.dram_tensor("pwT", [C, C], BF16, kind="ExternalInput")
    w1T_d = nc.dram_tensor("w1T", [C, C], E4, kind="ExternalInput")
    w2T_d = nc.dram_tensor("w2T", [C, C], E4, kind="ExternalInput")
    onesdr_d = nc.dram_tensor("onesdr", [128, 2, 16], E4, kind="ExternalInput")
    pb_d = nc.dram_tensor("pb", [128, CT], F32, kind="ExternalInput")
    b1_d = nc.dram_tensor("b1", [128, CT], F32, kind="ExternalInput")
    b2_d = nc.dram_tensor("b2", [128, CT], F32, kind="ExternalInput")
    eb_d = nc.dram_tensor("eb", [128, 1], F32, kind="ExternalInput")
    y_d = nc.dram_tensor("y", [ITEMS, 128, CT * N], BF16, kind="ExternalOutput")

    with tile.TileContext(nc) as tc:
        with (
            tc.tile_pool(name="const", bufs=1) as constp,
            tc.tile_pool(name="xin", bufs=2) as xp,
            tc.tile_pool(name="xtp", bufs=2) as xtp,
            tc.tile_pool(name="qt", bufs=2) as qp,
            tc.tile_pool(name="ebig", bufs=2) as ep,
            tc.tile_pool(name="aggp", bufs=2) as aggp,
            tc.tile_pool(name="htp", bufs=2) as hp,
            tc.tile_pool(name="yp", bufs=2) as yp,
            tc.tile_pool(name="statp", bufs=2) as statp,
            tc.tile_pool(name="psa", bufs=2, space=bass.MemorySpace.PSUM) as psa,
            tc.tile_pool(name="psb", bufs=3, space=bass.MemorySpace.PSUM) as psb,
            tc.tile_pool(name="psz", bufs=1, space=bass.MemorySpace.PSUM) as pszp,
        ):
            # ---- constants ----
            pwT = constp.tile([128, CT, C], BF16)
            w1T = constp.tile([128, CT, C], E4)
            w2T = constp.tile([128, CT, C], E4)
            onesdr = constp.tile([128, 2, 16], E4)
            pb = constp.tile([128, CT], F32)
            b1 = constp.tile([128, CT], F32)
            b2 = constp.tile([128, CT], F32)
            eb = constp.tile([128, 1], F32)

            # per-item tiles, allocated up front so emission can interleave
            Xbf = [xp.tile([128, CT, N], BF16, tag="X", name=f"X{i}")
                   for i in range(ITEMS)]
            XT8 = [xtp.tile([128, NT, C], E4, tag="XT", name=f"XT{i}")
                   for i in range(ITEMS)]
            qT8 = [qp.tile([128, CT, N], E4, tag="qT", name=f"qT{i}")
                   for i in range(ITEMS)]
            E5t = [ep.tile([128, NT, N], E5, tag="E", name=f"E{i}")
                   for i in range(ITEMS)]
            rrow = [statp.tile([1, N], F32, tag="rrow", name=f"rr{i}")
                    for i in range(ITEMS)]
            Rbs = [statp.tile([128, N], F32, tag="rbs", name=f"Rbs{i}")
                   for i in range(ITEMS)]
            agg8 = [aggp.tile([128, CT, N], E4, tag="aggT", name=f"agg{i}")
                    for i in range(ITEMS)]
            hT8 = [hp.tile([128, CT, N], E4, tag="hT", name=f"hT{i}")
                   for i in range(ITEMS)]
            Y = [yp.tile([128, CT, N], BF16, tag="Y", name=f"Y{i}")
                 for i in range(ITEMS)]
            pszs = [[pszp.tile([1, 512], F32, tag="psz", name=f"psz{i}_{m}")
                     for m in range(NF)] for i in range(ITEMS)]

            # ---- DMA: SP queue gets the critical path (pwT, item0) ----
            # SP/HWDGE queue: the startup-critical loads, in dependency order
            nc.sync.dma_start(pwT[:], pwT_d.ap().rearrange("(t p) m -> p t m", p=128))
            for nf in range(NF):
                for ct in range(CT):
                    nc.sync.dma_start(
                        Xbf[0][:, ct, ts(nf, 512)],
                        xbf_d.ap()[0][:, ts(ct * NF + nf, 512)],
                    )
            nc.sync.dma_start(XT8[0][:], xt8_d.ap()[0])
            nc.sync.dma_start(onesdr[:], onesdr_d.ap())
            # SWDGE (gpsimd) queue: small constants + item1 + pass-B weights
            nc.gpsimd.dma_start(pb[:], pb_d.ap())
            nc.gpsimd.dma_start(eb[:], eb_d.ap())
            nc.gpsimd.dma_start(Xbf[1][:], xbf_d.ap()[1])
            nc.gpsimd.dma_start(XT8[1][:], xt8_d.ap()[1])
            nc.gpsimd.dma_start(
                w1T[:], w1T_d.ap().rearrange("(t p) m -> p t m", p=128))
            nc.gpsimd.dma_start(
                w2T[:], w2T_d.ap().rearrange("(t p) m -> p t m", p=128))
            nc.gpsimd.dma_start(b1[:], b1_d.ap())
            nc.gpsimd.dma_start(b2[:], b2_d.ap())

            def proj(it):
                """qT = proj_w @ X + pb, emitted nf-major so S can start
                after the nf=0 bias chunks."""
                for nf in range(NF):
                    pss = [psb.tile([128, 512], F32, tag="mmb",
                                    name=f"q{it}_{mt}_{nf}") for mt in range(CT)]
                    for mt in range(CT):
                        for kt in range(CT):
                            nc.tensor.matmul(
                                pss[mt][:],
                                pwT[:, kt, ts(mt, 128)],
                                Xbf[it][:, kt, ts(nf, 512)],
                                start=(kt == 0),
                                stop=(kt == CT - 1),
                            )
                    for mt in range(CT):
                        if it == 0 and nf == 1:
                            nc.scalar.activation(
                                qT8[it][:, mt, ts(nf, 512)],
                                pss[mt][:],
                                AFT.Identity, bias=pb[:, mt : mt + 1],
                            )
                        else:
                            nc.vector.tensor_scalar(
                                qT8[it][:, mt, ts(nf, 512)],
                                pss[mt][:],
                                pb[:, mt : mt + 1], None, AluOpType.add,
                            )

            def s_exp(it, nt):
                """one row-block: S matmuls + exp; Z chain links after odd nt."""
                ps = psa.tile([128, N], F32, tag="mm", name=f"s{it}_{nt}")
                for mf in range(NF):
                    nc.tensor.matmul(
                        ps[:, ts(mf, 512)],
                        qT8[it][:, :, ts(nt, 128)],
                        qT8[it][:, :, ts(mf, 512)],
                        start=True, stop=True, perf_mode=DR,
                    )
                nc.scalar.activation(
                    E5t[it][:, nt, :], ps[:], AFT.Exp,
                    bias=eb[:, 0:1], scale=0.0625,
                )
                if nt % 2 == 1:
                    tp = nt // 2
                    with tc.high_priority():
                        for mf in range(NF):
                            nc.tensor.matmul(
                                pszs[it][mf][:],
                                onesdr[:, :, 0:1],
                                E5t[it][:, nt - 1 : nt + 1, ts(mf, 512)],
                                start=(tp == 0), stop=(tp == NT // 2 - 1),
                                perf_mode=DR, skip_group_check=True,
                            )

            def recip_bcast(it):
                for mf in range(NF):
                    nc.vector.reciprocal(
                        rrow[it][0:1, ts(mf, 512)], pszs[it][mf][:])
                    nc.gpsimd.partition_broadcast(
                        Rbs[it][:, ts(mf, 512)], rrow[it][0:1, ts(mf, 512)])

            def agg(it):
                """agg matmuls nf-major + DVE normalize to e4m3."""
                for nf in range(NF):
                    pss = [psb.tile([128, 512], F32, tag="mmb",
                                    name=f"g{it}_{ct}_{nf}") for ct in range(CT)]
                    for ct in range(CT):
                        for tp in range(NT // 2):
                            nc.tensor.matmul(
                                pss[ct][:],
                                XT8[it][:, 2 * tp : 2 * tp + 2, ts(ct, 128)],
                                E5t[it][:, 2 * tp : 2 * tp + 2, ts(nf, 512)],
                                start=(tp == 0), stop=(tp == NT // 2 - 1),
                                perf_mode=DR,
                            )
                    for ct in range(CT):
                        nc.vector.tensor_tensor(
                            agg8[it][:, ct, ts(nf, 512)],
                            pss[ct][:],
                            Rbs[it][:, ts(nf, 512)],
                            AluOpType.mult,
                        )

            def mlp1(it):
                for mt in range(CT):
                    for nf in range(NF):
                        ps = psb.tile([128, 512], F32, tag="mmb",
                                      name=f"h{it}_{mt}_{nf}")
                        nc.tensor.matmul(
                            ps[:],
                            w1T[:, :, ts(mt, 128)],
                            agg8[it][:, :, ts(nf, 512)],
                            start=True, stop=True, perf_mode=DR,
                        )
                        nc.scalar.activation(
                            hT8[it][:, mt, ts(nf, 512)],
                            ps[:], AFT.Gelu,
                            bias=b1[:, mt : mt + 1],
                        )

            def mlp2(it):
                for mt in range(CT):
                    for nf in range(NF):
                        ps = psb.tile([128, 512], F32, tag="mmb",
                                      name=f"o{it}_{mt}_{nf}")
                        nc.tensor.matmul(
                            ps[:],
                            w2T[:, :, ts(mt, 128)],
                            hT8[it][:, :, ts(nf, 512)],
                            start=True, stop=True, perf_mode=DR,
                        )
                        nc.vector.scalar_tensor_tensor(
                            Y[it][:, mt, ts(nf, 512)],
                            ps[:],
                            b2[:, mt : mt + 1],
                            Xbf[it][:, mt, ts(nf, 512)],
                            AluOpType.add, AluOpType.add,
                        )
                        nc.sync.dma_start(
                            y_d.ap()[it][:, ts(mt * NF + nf, 512)],
                            Y[it][:, mt, ts(nf, 512)],
                        )

            # ---- software-pipelined emission ----
            proj(0)
            for nt in range(5):
                s_exp(0, nt)
            proj(1)                      # overlaps exp0 on ACT
            for nt in range(5, NT):
                s_exp(0, nt)
            recip_bcast(0)
            s_exp(1, 0)
            s_exp(1, 1)
            agg(0)                       # E0 complete; runs during exp1
            for nt in range(2, NT):
                s_exp(1, nt)
            with tc.high_priority():
                recip_bcast(1)
            mlp1(0)
            mlp2(0)
            agg(1)
            mlp1(1)
            mlp2(1)

    nc.compile()
    return nc


_NC_CACHE = {}


def _get_nc():
    if "nc" not in _NC_CACHE:
        _NC_CACHE["nc"] = build_nc()
    return _NC_CACHE["nc"]


def make_in_maps(x, proj_w, proj_b, w1, b1, w2, b2):
    B = x.shape[0]
    xs = x.reshape(B, C, N)
    # p-major packing: [B, 128, CT*N], row p holds channels {p, 128+p}
    xbf = np.ascontiguousarray(
        xs.reshape(B, CT, 128, N).transpose(0, 2, 1, 3).reshape(B, 128, CT * N)
    ).astype(NP_BF)
    # XT p-major: [B, 128, NT*C], row p holds nodes {p, 128+p, ..., 896+p}
    xt8 = np.ascontiguousarray(
        xs.transpose(0, 2, 1).reshape(B, NT, 128, C).transpose(0, 2, 1, 3)
        .reshape(B, 128, NT * C)
    ).astype(NP_E4)
    shared = {
        "pwT": np.ascontiguousarray(proj_w.T).astype(NP_BF),
        "w1T": np.ascontiguousarray(w1.T).astype(NP_E4),
        "w2T": np.ascontiguousarray(w2.T).astype(NP_E4),
        "onesdr": np.ones((128, 2, 16), dtype=NP_E4),
        "pb": np.ascontiguousarray(proj_b.reshape(CT, 128).T.astype(np.float32)),
        "b1": np.ascontiguousarray(b1.reshape(CT, 128).T.astype(np.float32)),
        "b2": np.ascontiguousarray(b2.reshape(CT, 128).T.astype(np.float32)),
        "eb": np.full((128, 1), EXP_BIAS, dtype=np.float32),
    }
    in_maps = []
    for c in range(N_CORES):
        m = dict(shared)
        m["xbf"] = np.ascontiguousarray(xbf[c * ITEMS : (c + 1) * ITEMS])
        m["xt8"] = np.ascontiguousarray(xt8[c * ITEMS : (c + 1) * ITEMS])
        in_maps.append(m)
    return in_maps


def kernel(x, proj_w, proj_b, w1, b1, w2, b2, _trace=False, **trace_kw):
    nc = _get_nc()
    in_maps = make_in_maps(x, proj_w, proj_b, w1, b1, w2, b2)
    res = run_bass_kernel_spmd(
        nc, in_maps, list(range(N_CORES)), trace=_trace, **trace_kw
    )
    B, _, H, W = x.shape
    # unpack p-major bf16 [128, CT*N] -> [C, N] f32
    outs = [
        np.asarray(r["y"]).astype(np.float32)
        .reshape(ITEMS, 128, CT, N).transpose(0, 2, 1, 3).reshape(ITEMS, C, N)
        for r in res.results
    ]
    y = np.concatenate(outs, axis=0).reshape(B, C, H, W).astype(np.float32)
    if _trace:
        kernel.last_result = res
    return y


# revision 17
# speedup vs baseline: 1.0383x; 1.0383x over previous
"""Trainium2 Bass kernel for a dense graph-transformer block (fp8 DoubleRow).

Reference computation (per batch item b, with C=256, N=H*W=1024):
    nodes = x[b].reshape(C, N).T                      # [N, C]
    q     = nodes @ proj_w.T + proj_b                 # [N, C]
    S     = (q @ q.T) / sqrt(C)                       # [N, N]  (symmetric!)
    A     = softmax(S, axis=-1)
    agg   = A @ nodes                                 # [N, C]
    h     = gelu(agg @ w1.T + b1)  (erf gelu)
    out   = h @ w2.T + b2
    y[b]  = x[b] + out.T.reshape(C, H, W)

Strategy (data-parallel over batch, 2 items per core, 8 cores):
  Everything stays in the transposed layout [C-on-partitions, N-free].
  All heavy matmuls run in fp8 DoubleRow perf mode (2 k-tiles folded per
  instruction, 0.5 cycles per output row):
    - qT   = proj_w @ X + pb      (bf16 matmul; DVE adds bias, emits e4m3)
    - S    = qT.T @ qT            (e4m3 x e4m3 DoubleRow)
    - E    = exp(S/16 - 4.5)      (ACT, emits e5m2; e5m2's wide exponent
                                   absorbs the full score range with a single
                                   global shift that cancels in softmax)
    - Z    = ones.T @ E           (e4m3 ones x e5m2 E DoubleRow; E symmetric
                                   so column sums equal row sums)
    - agg  = XT.T @ E             (e4m3 XT x e5m2 E DoubleRow), scaled by
                                   broadcast(1/Z) (gpsimd partition_broadcast)
    - mlp  = two more e4m3 DoubleRow layers; gelu fused on ACT.
  XT ships pre-transposed from the host (layout prep, zero flops), as do the
  fp8/bf16 weight casts; all big tensors ship p-major packed so each DMA is
  128 descriptors of >=2KB.  The two items are software-pipelined phase by
  phase so ACT (the bottleneck: 16 exps + 4 gelus) never starves, and ACT
  switches tables (exp<->gelu) only twice per launch.
"""

import os
import sys

import numpy as np

for _p in ("/opt/trn_rl_repo", "/root/.axon_site/_ro/trn_rl_repo"):
    if os.path.isdir(_p) and _p not in sys.path:
        sys.path.insert(0, _p)

import ml_dtypes
import concourse.bass as bass
import concourse.bacc as bacc
import concourse.mybir as mybir
from concourse import tile
from concourse.alu_op_type import AluOpType
from concourse.bass_utils import run_bass_kernel_spmd

F32 = mybir.dt.float32
BF16 = mybir.dt.bfloat16
E4 = mybir.dt.float8e4
E5 = mybir.dt.float8e5
AFT = mybir.ActivationFunctionType
DR = mybir.MatmulPerfMode.DoubleRow

NP_E4 = ml_dtypes.float8_e4m3
NP_E5 = ml_dtypes.float8_e5m2
NP_BF = ml_dtypes.bfloat16

C = 256          # channels
N = 1024         # nodes = H*W
CT = C // 128    # channel partition-tiles (2)
NT = N // 128    # node partition-tiles (8)
NF = N // 512    # node free-chunks of 512 (2)
N_CORES = 8
ITEMS = 2        # batch items per core (B=16 / 8 cores)
EXP_BIAS = -4.5  # global exp shift; cancels in softmax, keeps E in e5m2 range


def ts(i, size):
    return slice(i * size, (i + 1) * size)


def build_nc():
    nc = bacc.Bacc(None, target_bir_lowering=False)

    xbf_d = nc.dram_tensor("xbf", [ITEMS, 128, CT * N], BF16, kind="ExternalInput")
    xt8_d = nc.dram_tensor("xt8", [ITEMS, 128, NT * C], E4, kind="ExternalInput")
    pwT_d = nc.dram_tensor("pwT", [C, C], BF16, kind="ExternalInput")
    w1T_d = nc.dram_tensor("w1T", [C, C], E4, kind="ExternalInput")
    w2T_d = nc.dram_tensor("w2T", [C, C], E4, kind="ExternalInput")
    onesdr_d = nc.dram_tensor("onesdr", [128, 2, 16], E4, kind="ExternalInput")
    pb_d = nc.dram_tensor("pb", [128, CT], F32, kind="ExternalInput")
    b1_d = nc.dram_tensor("b1", [128, CT], F32, kind="ExternalInput")
    b2_d = nc.dram_tensor("b2", [128, CT], F32, kind="ExternalInput")
    eb_d = nc.dram_tensor("eb", [128, 1], F32, kind="ExternalInput")
    y_d = nc.dram_tensor("y", [ITEMS, 128, CT * N], BF16, kind="ExternalOutput")

    with tile.TileContext(nc) as tc:
        with (
            tc.tile_pool(name="const", bufs=1) as constp,
            tc.tile_pool(name="xin", bufs=2) as xp,
            tc.tile_pool(name="xtp", bufs=2) as xtp,
            tc.tile_pool(name="qt", bufs=2) as qp,
            tc.tile_pool(name="ebig", bufs=2) as ep,
            tc.tile_pool(name="aggp", bufs=2) as aggp,
            tc.tile_pool(name="htp", bufs=2) as hp,
            tc.tile_pool(name="yp", bufs=2) as yp,
            tc.tile_pool(name="statp", bufs=2) as statp,
            tc.tile_pool(name="psa", bufs=2, space=bass.MemorySpace.PSUM) as psa,
            tc.tile_pool(name="psb", bufs=3, space=bass.MemorySpace.PSUM) as psb,
            tc.tile_pool(name="psz", bufs=1, space=bass.MemorySpace.PSUM) as pszp,
        ):
            # ---- constants ----
            pwT = constp.tile([128, CT, C], BF16)
            w1T = constp.tile([128, CT, C], E4)
            w2T = constp.tile([128, CT, C], E4)
            onesdr = constp.tile([128, 2, 16], E4)
            pb = constp.tile([128, CT], F32)
            b1 = constp.tile([128, CT], F32)
            b2 = constp.tile([128, CT], F32)
            eb = constp.tile([128, 1], F32)

            # per-item tiles, allocated up front so emission can interleave
            Xbf = [xp.tile([128, CT, N], BF16, tag="X", name=f"X{i}")
                   for i in range(ITEMS)]
            XT8 = [xtp.tile([128, NT, C], E4, tag="XT", name=f"XT{i}")
                   for i in range(ITEMS)]
            qT8 = [qp.tile([128, CT, N], E4, tag="qT", name=f"qT{i}")
                   for i in range(ITEMS)]
            E5t = [ep.tile([128, NT, N], E5, tag="E", name=f"E{i}")
                   for i in range(ITEMS)]
            rrow = [statp.tile([1, N], F32, tag="rrow", name=f"rr{i}")
                    for i in range(ITEMS)]
            Rbs = [statp.tile([128, N], F32, tag="rbs", name=f"Rbs{i}")
                   for i in range(ITEMS)]
            agg8 = [aggp.tile([128, CT, N], E4, tag="aggT", name=f"agg{i}")
                    for i in range(ITEMS)]
            hT8 = [hp.tile([128, CT, N], E4, tag="hT", name=f"hT{i}")
                   for i in range(ITEMS)]
            Y = [yp.tile([128, CT, N], BF16, tag="Y", name=f"Y{i}")
                 for i in range(ITEMS)]
            pszs = [[pszp.tile([1, 512], F32, tag="psz", name=f"psz{i}_{m}")
                     for m in range(NF)] for i in range(ITEMS)]

            # ---- DMA: SP queue gets the critical path (pwT, item0) ----
            # SP/HWDGE queue: the startup-critical loads, in dependency order
            nc.sync.dma_start(pwT[:], pwT_d.ap().rearrange("(t p) m -> p t m", p=128))
            nc.sync.dma_start(Xbf[0][:], xbf_d.ap()[0])
            nc.sync.dma_start(XT8[0][:], xt8_d.ap()[0])
            nc.sync.dma_start(onesdr[:], onesdr_d.ap())
            # SWDGE (gpsimd) queue: small constants + item1 + pass-B weights
            nc.gpsimd.dma_start(pb[:], pb_d.ap())
            nc.gpsimd.dma_start(eb[:], eb_d.ap())
            nc.gpsimd.dma_start(Xbf[1][:], xbf_d.ap()[1])
            nc.gpsimd.dma_start(XT8[1][:], xt8_d.ap()[1])
            nc.gpsimd.dma_start(
                w1T[:], w1T_d.ap().rearrange("(t p) m -> p t m", p=128))
            nc.gpsimd.dma_start(
                w2T[:], w2T_d.ap().rearrange("(t p) m -> p t m", p=128))
            nc.gpsimd.dma_start(b1[:], b1_d.ap())
            nc.gpsimd.dma_start(b2[:], b2_d.ap())

            def proj(it):
                """qT = proj_w @ X + pb, emitted nf-major so S can start
                after the nf=0 bias chunks."""
                for nf in range(NF):
                    pss = [psb.tile([128, 512], F32, tag="mmb",
                                    name=f"q{it}_{mt}_{nf}") for mt in range(CT)]
                    for mt in range(CT):
                        for kt in range(CT):
                            nc.tensor.matmul(
                                pss[mt][:],
                                pwT[:, kt, ts(mt, 128)],
                                Xbf[it][:, kt, ts(nf, 512)],
                                start=(kt == 0),
                                stop=(kt == CT - 1),
                            )
                    for mt in range(CT):
                        if it == 0 and nf == 1:
                            nc.scalar.activation(
                                qT8[it][:, mt, ts(nf, 512)],
                                pss[mt][:],
                                AFT.Identity, bias=pb[:, mt : mt + 1],
                            )
                        else:
                            nc.vector.tensor_scalar(
                                qT8[it][:, mt, ts(nf, 512)],
                                pss[mt][:],
                                pb[:, mt : mt + 1], None, AluOpType.add,
                            )

            def s_exp(it, nt):
                """one row-block: S matmuls + exp; Z chain links after odd nt."""
                ps = psa.tile([128, N], F32, tag="mm", name=f"s{it}_{nt}")
                for mf in range(NF):
                    nc.tensor.matmul(
                        ps[:, ts(mf, 512)],
                        qT8[it][:, :, ts(nt, 128)],
                        qT8[it][:, :, ts(mf, 512)],
                        start=True, stop=True, perf_mode=DR,
                    )
                nc.scalar.activation(
                    E5t[it][:, nt, :], ps[:], AFT.Exp,
                    bias=eb[:, 0:1], scale=0.0625,
                )
                if nt % 2 == 1:
                    tp = nt // 2
                    with tc.high_priority():
                        for mf in range(NF):
                            nc.tensor.matmul(
                                pszs[it][mf][:],
                                onesdr[:, :, 0:1],
                                E5t[it][:, nt - 1 : nt + 1, ts(mf, 512)],
                                start=(tp == 0), stop=(tp == NT // 2 - 1),
                                perf_mode=DR, skip_group_check=True,
                            )

            def recip_bcast(it):
                for mf in range(NF):
                    nc.vector.reciprocal(
                        rrow[it][0:1, ts(mf, 512)], pszs[it][mf][:])
                    nc.gpsimd.partition_broadcast(
                        Rbs[it][:, ts(mf, 512)], rrow[it][0:1, ts(mf, 512)])

            def agg(it, big=False):
                """agg matmuls nf-major + DVE normalize to e4m3.  big=True
                uses the psa pool (free after the last exp) so the final
                item's agg is not gated by the psb mlp rotation."""
                if big:
                    pss = [psa.tile([128, N], F32, tag="mm", name=f"g{it}_{ct}")
                           for ct in range(CT)]
                    get = lambda ct, nf: pss[ct][:, ts(nf, 512)]
                else:
                    tiles = {}
                    def get(ct, nf):
                        if (ct, nf) not in tiles:
                            tiles[(ct, nf)] = psb.tile(
                                [128, 512], F32, tag="mmb", name=f"g{it}_{ct}_{nf}")
                        return tiles[(ct, nf)][:]
                for nf in range(NF):
                    for ct in range(CT):
                        for tp in range(NT // 2):
                            nc.tensor.matmul(
                                get(ct, nf),
                                XT8[it][:, 2 * tp : 2 * tp + 2, ts(ct, 128)],
                                E5t[it][:, 2 * tp : 2 * tp + 2, ts(nf, 512)],
                                start=(tp == 0), stop=(tp == NT // 2 - 1),
                                perf_mode=DR,
                            )
                    for ct in range(CT):
                        with tc.high_priority(offset=100 if big else None):
                            nc.vector.tensor_tensor(
                                agg8[it][:, ct, ts(nf, 512)],
                                get(ct, nf),
                                Rbs[it][:, ts(nf, 512)],
                                AluOpType.mult,
                            )

            def mlp1(it):
                for mt in range(CT):
                    for nf in range(NF):
                        ps = psb.tile([128, 512], F32, tag="mmb",
                                      name=f"h{it}_{mt}_{nf}")
                        nc.tensor.matmul(
                            ps[:],
                            w1T[:, :, ts(mt, 128)],
                            agg8[it][:, :, ts(nf, 512)],
                            start=True, stop=True, perf_mode=DR,
                        )
                        nc.scalar.activation(
                            hT8[it][:, mt, ts(nf, 512)],
                            ps[:], AFT.Gelu,
                            bias=b1[:, mt : mt + 1],
                        )

            def mlp2(it):
                for mt in range(CT):
                    for nf in range(NF):
                        ps = psb.tile([128, 512], F32, tag="mmb",
                                      name=f"o{it}_{mt}_{nf}")
                        nc.tensor.matmul(
                            ps[:],
                            w2T[:, :, ts(mt, 128)],
                            hT8[it][:, :, ts(nf, 512)],
                            start=True, stop=True, perf_mode=DR,
                        )
                        nc.vector.scalar_tensor_tensor(
                            Y[it][:, mt, ts(nf, 512)],
                            ps[:],
                            b2[:, mt : mt + 1],
                            Xbf[it][:, mt, ts(nf, 512)],
                            AluOpType.add, AluOpType.add,
                        )
                        nc.sync.dma_start(
                            y_d.ap()[it][:, ts(mt * NF + nf, 512)],
                            Y[it][:, mt, ts(nf, 512)],
                        )

            # ---- software-pipelined emission ----
            proj(0)
            for nt in range(5):
                s_exp(0, nt)
            proj(1)                      # overlaps exp0 on ACT
            for nt in range(5, NT):
                s_exp(0, nt)
            recip_bcast(0)
            s_exp(1, 0)
            s_exp(1, 1)
            agg(0)                       # E0 complete; runs during exp1
            for nt in range(2, NT):
                s_exp(1, nt)
            with tc.high_priority():
                recip_bcast(1)
            agg(1, big=True)
            mlp1(0)
            mlp2(0)
            mlp1(1)
            mlp2(1)

    nc.compile()
    return nc


_NC_CACHE = {}


def _get_nc():
    if "nc" not in _NC_CACHE:
        _NC_CACHE["nc"] = build_nc()
    return _NC_CACHE["nc"]


def make_in_maps(x, proj_w, proj_b, w1, b1, w2, b2):
    B = x.shape[0]
    xs = x.reshape(B, C, N)
    # p-major packing: [B, 128, CT*N], row p holds channels {p, 128+p}
    xbf = np.ascontiguousarray(
        xs.reshape(B, CT, 128, N).transpose(0, 2, 1, 3).reshape(B, 128, CT * N)
    ).astype(NP_BF)
    # XT p-major: [B, 128, NT*C], row p holds nodes {p, 128+p, ..., 896+p}
    xt8 = np.ascontiguousarray(
        xs.transpose(0, 2, 1).reshape(B, NT, 128, C).transpose(0, 2, 1, 3)
        .reshape(B, 128, NT * C)
    ).astype(NP_E4)
    shared = {
        "pwT": np.ascontiguousarray(proj_w.T).astype(NP_BF),
        "w1T": np.ascontiguousarray(w1.T).astype(NP_E4),
        "w2T": np.ascontiguousarray(w2.T).astype(NP_E4),
        "onesdr": np.ones((128, 2, 16), dtype=NP_E4),
        "pb": np.ascontiguousarray(proj_b.reshape(CT, 128).T.astype(np.float32)),
        "b1": np.ascontiguousarray(b1.reshape(CT, 128).T.astype(np.float32)),
        "b2": np.ascontiguousarray(b2.reshape(CT, 128).T.astype(np.float32)),
        "eb": np.full((128, 1), EXP_BIAS, dtype=np.float32),
    }
    in_maps = []
    for c in range(N_CORES):
        m = dict(shared)
        m["xbf"] = np.ascontiguousarray(xbf[c * ITEMS : (c + 1) * ITEMS])
        m["xt8"] = np.ascontiguousarray(xt8[c * ITEMS : (c + 1) * ITEMS])
        in_maps.append(m)
    return in_maps


def kernel(x, proj_w, proj_b, w1, b1, w2, b2, _trace=False, **trace_kw):
    nc = _get_nc()
    in_maps = make_in_maps(x, proj_w, proj_b, w1, b1, w2, b2)
    res = run_bass_kernel_spmd(
        nc, in_maps, list(range(N_CORES)), trace=_trace, **trace_kw
    )
    B, _, H, W = x.shape
    # unpack p-major bf16 [128, CT*N] -> [C, N] f32
    outs = [
        np.asarray(r["y"]).astype(np.float32)
        .reshape(ITEMS, 128, CT, N).transpose(0, 2, 1, 3).reshape(ITEMS, C, N)
        for r in res.results
    ]
    y = np.concatenate(outs, axis=0).reshape(B, C, H, W).astype(np.float32)
    if _trace:
        kernel.last_result = res
    return y


# revision 18
# speedup vs baseline: 1.0773x; 1.0375x over previous
"""Trainium2 Bass kernel for a dense graph-transformer block (fp8 DoubleRow).

Reference computation (per batch item b, with C=256, N=H*W=1024):
    nodes = x[b].reshape(C, N).T                      # [N, C]
    q     = nodes @ proj_w.T + proj_b                 # [N, C]
    S     = (q @ q.T) / sqrt(C)                       # [N, N]  (symmetric!)
    A     = softmax(S, axis=-1)
    agg   = A @ nodes                                 # [N, C]
    h     = gelu(agg @ w1.T + b1)  (erf gelu)
    out   = h @ w2.T + b2
    y[b]  = x[b] + out.T.reshape(C, H, W)

Strategy (data-parallel over batch, 2 items per core, 8 cores):
  Everything stays in the transposed layout [C-on-partitions, N-free].
  All heavy matmuls run in fp8 DoubleRow perf mode (2 k-tiles folded per
  instruction, 0.5 cycles per output row):
    - qT   = proj_w @ X + pb      (bf16 matmul; DVE adds bias, emits e4m3)
    - S    = qT.T @ qT            (e4m3 x e4m3 DoubleRow)
    - E    = exp(S/16 - 4.5)      (ACT, emits e5m2; e5m2's wide exponent
                                   absorbs the full score range with a single
                                   global shift that cancels in softmax)
    - Z    = ones.T @ E           (e4m3 ones x e5m2 E DoubleRow; E symmetric
                                   so column sums equal row sums)
    - agg  = XT.T @ E             (e4m3 XT x e5m2 E DoubleRow), scaled by
                                   broadcast(1/Z) (gpsimd partition_broadcast)
    - mlp  = two more e4m3 DoubleRow layers; gelu fused on ACT.
  XT ships pre-transposed from the host (layout prep, zero flops), as do the
  fp8/bf16 weight casts; all big tensors ship p-major packed so each DMA is
  128 descriptors of >=2KB.  The two items are software-pipelined phase by
  phase so ACT (the bottleneck: 16 exps + 4 gelus) never starves, and ACT
  switches tables (exp<->gelu) only twice per launch.
"""

import os
import sys

import numpy as np

for _p in ("/opt/trn_rl_repo", "/root/.axon_site/_ro/trn_rl_repo"):
    if os.path.isdir(_p) and _p not in sys.path:
        sys.path.insert(0, _p)

import ml_dtypes
import concourse.bass as bass
import concourse.bacc as bacc
import concourse.mybir as mybir
from concourse import tile
from concourse.alu_op_type import AluOpType
from concourse.bass_utils import run_bass_kernel_spmd

F32 = mybir.dt.float32
BF16 = mybir.dt.bfloat16
E4 = mybir.dt.float8e4
E5 = mybir.dt.float8e5
AFT = mybir.ActivationFunctionType
DR = mybir.MatmulPerfMode.DoubleRow

NP_E4 = ml_dtypes.float8_e4m3
NP_E5 = ml_dtypes.float8_e5m2
NP_BF = ml_dtypes.bfloat16

C = 256          # channels
N = 1024         # nodes = H*W
CT = C // 128    # channel partition-tiles (2)
NT = N // 128    # node partition-tiles (8)
NF = N // 512    # node free-chunks of 512 (2)
N_CORES = 8
ITEMS = 2        # batch items per core (B=16 / 8 cores)
EXP_BIAS = -4.5  # global exp shift; cancels in softmax, keeps E in e5m2 range


def ts(i, size):
    return slice(i * size, (i + 1) * size)


def build_nc():
    nc = bacc.Bacc(None, target_bir_lowering=False)

    xbf_d = nc.dram_tensor("xbf", [ITEMS, 128, CT * N], BF16, kind="ExternalInput")
    xt8_d = nc.dram_tensor("xt8", [ITEMS, 128, NT * C], E4, kind="ExternalInput")
    pwT_d = nc.dram_tensor("pwT", [C, C], BF16, kind="ExternalInput")
    w1T_d = nc.dram_tensor("w1T", [C, C], E4, kind="ExternalInput")
    w2T_d = nc.dram_tensor("w2T", [C, C], E4, kind="ExternalInput")
    onesdr_d = nc.dram_tensor("onesdr", [128, 2, 16], E4, kind="ExternalInput")
    pb_d = nc.dram_tensor("pb", [128, CT], F32, kind="ExternalInput")
    b1_d = nc.dram_tensor("b1", [128, CT], F32, kind="ExternalInput")
    b2_d = nc.dram_tensor("b2", [128, CT], F32, kind="ExternalInput")
    eb_d = nc.dram_tensor("eb", [128, 1], F32, kind="ExternalInput")
    y_d = nc.dram_tensor("y", [ITEMS, 128, CT * N], BF16, kind="ExternalOutput")

    with tile.TileContext(nc) as tc:
        with (
            tc.tile_pool(name="const", bufs=1) as constp,
            tc.tile_pool(name="xin", bufs=2) as xp,
            tc.tile_pool(name="xtp", bufs=2) as xtp,
            tc.tile_pool(name="qt", bufs=2) as qp,
            tc.tile_pool(name="ebig", bufs=2) as ep,
            tc.tile_pool(name="aggp", bufs=2) as aggp,
            tc.tile_pool(name="htp", bufs=2) as hp,
            tc.tile_pool(name="yp", bufs=2) as yp,
            tc.tile_pool(name="statp", bufs=2) as statp,
            tc.tile_pool(name="psa", bufs=2, space=bass.MemorySpace.PSUM) as psa,
            tc.tile_pool(name="psb", bufs=4, space=bass.MemorySpace.PSUM) as psb,
        ):
            # ---- constants ----
            pwT = constp.tile([128, CT, C], BF16)
            w1T = constp.tile([128, CT, C], E4)
            w2T = constp.tile([128, CT, C], E4)
            onesdr = constp.tile([128, 2, 16], E4)
            pb = constp.tile([128, CT], F32)
            b1 = constp.tile([128, CT], F32)
            b2 = constp.tile([128, CT], F32)
            eb = constp.tile([128, 1], F32)

            # per-item tiles, allocated up front so emission can interleave
            Xbf = [xp.tile([128, CT, N], BF16, tag="X", name=f"X{i}")
                   for i in range(ITEMS)]
            XT8 = [xtp.tile([128, NT, C], E4, tag="XT", name=f"XT{i}")
                   for i in range(ITEMS)]
            qT8 = [qp.tile([128, CT, N], E4, tag="qT", name=f"qT{i}")
                   for i in range(ITEMS)]
            E5t = [ep.tile([128, NT, N], E5, tag="E", name=f"E{i}")
                   for i in range(ITEMS)]
            rrow = [statp.tile([1, N], F32, tag="rrow", name=f"rr{i}")
                    for i in range(ITEMS)]
            Rbs = [statp.tile([128, N], F32, tag="rbs", name=f"Rbs{i}")
                   for i in range(ITEMS)]
            agg8 = [aggp.tile([128, CT, N], E4, tag="aggT", name=f"agg{i}")
                    for i in range(ITEMS)]
            hT8 = [hp.tile([128, CT, N], E4, tag="hT", name=f"hT{i}")
                   for i in range(ITEMS)]
            Y = [yp.tile([128, CT, N], BF16, tag="Y", name=f"Y{i}")
                 for i in range(ITEMS)]


            # ---- DMA: SP queue gets the critical path (pwT, item0) ----
            # SP/HWDGE queue: the startup-critical loads, in dependency order
            nc.sync.dma_start(pwT[:], pwT_d.ap().rearrange("(t p) m -> p t m", p=128))
            nc.sync.dma_start(Xbf[0][:], xbf_d.ap()[0])
            nc.sync.dma_start(XT8[0][:], xt8_d.ap()[0])
            nc.sync.dma_start(onesdr[:], onesdr_d.ap())
            # SWDGE (gpsimd) queue: small constants + item1 + pass-B weights
            nc.gpsimd.dma_start(pb[:], pb_d.ap())
            nc.gpsimd.dma_start(eb[:], eb_d.ap())
            nc.gpsimd.dma_start(Xbf[1][:], xbf_d.ap()[1])
            nc.gpsimd.dma_start(XT8[1][:], xt8_d.ap()[1])
            nc.gpsimd.dma_start(
                w1T[:], w1T_d.ap().rearrange("(t p) m -> p t m", p=128))
            nc.gpsimd.dma_start(
                w2T[:], w2T_d.ap().rearrange("(t p) m -> p t m", p=128))
            nc.gpsimd.dma_start(b1[:], b1_d.ap())
            nc.gpsimd.dma_start(b2[:], b2_d.ap())

            def proj(it):
                """qT = proj_w @ X + pb, emitted nf-major so S can start
                after the nf=0 bias chunks."""
                for nf in range(NF):
                    pss = [psb.tile([128, 512], F32, tag="mmb",
                                    name=f"q{it}_{mt}_{nf}") for mt in range(CT)]
                    for mt in range(CT):
                        for kt in range(CT):
                            nc.tensor.matmul(
                                pss[mt][:],
                                pwT[:, kt, ts(mt, 128)],
                                Xbf[it][:, kt, ts(nf, 512)],
                                start=(kt == 0),
                                stop=(kt == CT - 1),
                            )
                    for mt in range(CT):
                        if it == 0 and nf == 1:
                            nc.scalar.activation(
                                qT8[it][:, mt, ts(nf, 512)],
                                pss[mt][:],
                                AFT.Identity, bias=pb[:, mt : mt + 1],
                            )
                        else:
                            nc.vector.tensor_scalar(
                                qT8[it][:, mt, ts(nf, 512)],
                                pss[mt][:],
                                pb[:, mt : mt + 1], None, AluOpType.add,
                            )

            def s_exp(it, nt):
                """one row-block: S matmuls + exp; Z chain links after odd nt."""
                ps = psa.tile([128, N], F32, tag="mm", name=f"s{it}_{nt}")
                for mf in range(NF):
                    nc.tensor.matmul(
                        ps[:, ts(mf, 512)],
                        qT8[it][:, :, ts(nt, 128)],
                        qT8[it][:, :, ts(mf, 512)],
                        start=True, stop=True, perf_mode=DR,
                    )
                nc.scalar.activation(
                    E5t[it][:, nt, :], ps[:], AFT.Exp,
                    bias=eb[:, 0:1], scale=0.0625,
                )

            def recip_bcast(it):
                """Z column sums (short psb-resident chains), then 1/Z and
                its partition-broadcast, per 512-chunk."""
                zt = [psb.tile([1, 512], F32, tag="mmb", name=f"z{it}_{m}")
                      for m in range(NF)]
                for tp in range(NT // 2):
                    for mf in range(NF):
                        nc.tensor.matmul(
                            zt[mf][:],
                            onesdr[:, :, 0:1],
                            E5t[it][:, 2 * tp : 2 * tp + 2, ts(mf, 512)],
                            start=(tp == 0), stop=(tp == NT // 2 - 1),
                            perf_mode=DR, skip_group_check=True,
                        )
                for mf in range(NF):
                    nc.vector.reciprocal(
                        rrow[it][0:1, ts(mf, 512)], zt[mf][:])
                    nc.gpsimd.partition_broadcast(
                        Rbs[it][:, ts(mf, 512)], rrow[it][0:1, ts(mf, 512)])

            def agg(it, big=False):
                """agg matmuls nf-major + DVE normalize to e4m3.  big=True
                uses the psa pool (free after the last exp) so the final
                item's agg is not gated by the psb mlp rotation."""
                if big:
                    pss = [psa.tile([128, N], F32, tag="mm", name=f"g{it}_{ct}")
                           for ct in range(CT)]
                    get = lambda ct, nf: pss[ct][:, ts(nf, 512)]
                else:
                    tiles = {}
                    def get(ct, nf):
                        if (ct, nf) not in tiles:
                            tiles[(ct, nf)] = psb.tile(
                                [128, 512], F32, tag="mmb", name=f"g{it}_{ct}_{nf}")
                        return tiles[(ct, nf)][:]
                for nf in range(NF):
                    for ct in range(CT):
                        for tp in range(NT // 2):
                            nc.tensor.matmul(
                                get(ct, nf),
                                XT8[it][:, 2 * tp : 2 * tp + 2, ts(ct, 128)],
                                E5t[it][:, 2 * tp : 2 * tp + 2, ts(nf, 512)],
                                start=(tp == 0), stop=(tp == NT // 2 - 1),
                                perf_mode=DR,
                            )
                    for ct in range(CT):
                        with tc.high_priority(offset=100 if big else None):
                            nc.vector.tensor_tensor(
                                agg8[it][:, ct, ts(nf, 512)],
                                get(ct, nf),
                                Rbs[it][:, ts(nf, 512)],
                                AluOpType.mult,
                            )

            def mlp1(it):
                for mt in range(CT):
                    for nf in range(NF):
                        ps = psb.tile([128, 512], F32, tag="mmb",
                                      name=f"h{it}_{mt}_{nf}")
                        nc.tensor.matmul(
                            ps[:],
                            w1T[:, :, ts(mt, 128)],
                            agg8[it][:, :, ts(nf, 512)],
                            start=True, stop=True, perf_mode=DR,
                        )
                        nc.scalar.activation(
                            hT8[it][:, mt, ts(nf, 512)],
                            ps[:], AFT.Gelu,
                            bias=b1[:, mt : mt + 1],
                        )

            def mlp2(it):
                for mt in range(CT):
                    for nf in range(NF):
                        ps = psb.tile([128, 512], F32, tag="mmb",
                                      name=f"o{it}_{mt}_{nf}")
                        nc.tensor.matmul(
                            ps[:],
                            w2T[:, :, ts(mt, 128)],
                            hT8[it][:, :, ts(nf, 512)],
                            start=True, stop=True, perf_mode=DR,
                        )
                        nc.vector.scalar_tensor_tensor(
                            Y[it][:, mt, ts(nf, 512)],
                            ps[:],
                            b2[:, mt : mt + 1],
                            Xbf[it][:, mt, ts(nf, 512)],
                            AluOpType.add, AluOpType.add,
                        )
                        nc.sync.dma_start(
                            y_d.ap()[it][:, ts(mt * NF + nf, 512)],
                            Y[it][:, mt, ts(nf, 512)],
                        )

            # ---- software-pipelined emission ----
            proj(0)
            for nt in range(5):
                s_exp(0, nt)
            proj(1)                      # overlaps exp0 on ACT
            for nt in range(5, NT):
                s_exp(0, nt)
            recip_bcast(0)
            s_exp(1, 0)
            s_exp(1, 1)
            agg(0)                       # E0 complete; runs during exp1
            for nt in range(2, NT):
                s_exp(1, nt)
            with tc.high_priority():
                recip_bcast(1)
            agg(1, big=True)
            mlp1(0)
            mlp2(0)
            mlp1(1)
            mlp2(1)

    nc.compile()
    return nc


_NC_CACHE = {}


def _get_nc():
    if "nc" not in _NC_CACHE:
        _NC_CACHE["nc"] = build_nc()
    return _NC_CACHE["nc"]


def make_in_maps(x, proj_w, proj_b, w1, b1, w2, b2):
    B = x.shape[0]
    xs = x.reshape(B, C, N)
    # p-major packing: [B, 128, CT*N], row p holds channels {p, 128+p}
    xbf = np.ascontiguousarray(
        xs.reshape(B, CT, 128, N).transpose(0, 2, 1, 3).reshape(B, 128, CT * N)
    ).astype(NP_BF)
    # XT p-major: [B, 128, NT*C], row p holds nodes {p, 128+p, ..., 896+p}
    xt8 = np.ascontiguousarray(
        xs.transpose(0, 2, 1).reshape(B, NT, 128, C).transpose(0, 2, 1, 3)
        .reshape(B, 128, NT * C)
    ).astype(NP_E4)
    shared = {
        "pwT": np.ascontiguousarray(proj_w.T).astype(NP_BF),
        "w1T": np.ascontiguousarray(w1.T).astype(NP_E4),
        "w2T": np.ascontiguousarray(w2.T).astype(NP_E4),
        "onesdr": np.ones((128, 2, 16), dtype=NP_E4),
        "pb": np.ascontiguousarray(proj_b.reshape(CT, 128).T.astype(np.float32)),
        "b1": np.ascontiguousarray(b1.reshape(CT, 128).T.astype(np.float32)),
        "b2": np.ascontiguousarray(b2.reshape(CT, 128).T.astype(np.float32)),
        "eb": np.full((128, 1), EXP_BIAS, dtype=np.float32),
    }
    in_maps = []
    for c in range(N_CORES):
        m = dict(shared)
        m["xbf"] = np.ascontiguousarray(xbf[c * ITEMS : (c + 1) * ITEMS])
        m["xt8"] = np.ascontiguousarray(xt8[c * ITEMS : (c + 1) * ITEMS])
        in_maps.append(m)
    return in_maps


def kernel(x, proj_w, proj_b, w1, b1, w2, b2, _trace=False, **trace_kw):
    nc = _get_nc()
    in_maps = make_in_maps(x, proj_w, proj_b, w1, b1, w2, b2)
    res = run_bass_kernel_spmd(
        nc, in_maps, list(range(N_CORES)), trace=_trace, **trace_kw
    )
    B, _, H, W = x.shape
    # unpack p-major bf16 [128, CT*N] -> [C, N] f32
    outs = [
        np.asarray(r["y"]).astype(np.float32)
        .reshape(ITEMS, 128, CT, N).transpose(0, 2, 1, 3).reshape(ITEMS, C, N)
        for r in res.results
    ]
    y = np.concatenate(outs, axis=0).reshape(B, C, H, W).astype(np.float32)
    if _trace:
        kernel.last_result = res
    return y


# revision 20
# speedup vs baseline: 1.0838x; 1.0061x over previous
"""Trainium2 Bass kernel for a dense graph-transformer block (fp8 DoubleRow).

Reference computation (per batch item b, with C=256, N=H*W=1024):
    nodes = x[b].reshape(C, N).T                      # [N, C]
    q     = nodes @ proj_w.T + proj_b                 # [N, C]
    S     = (q @ q.T) / sqrt(C)                       # [N, N]  (symmetric!)
    A     = softmax(S, axis=-1)
    agg   = A @ nodes                                 # [N, C]
    h     = gelu(agg @ w1.T + b1)  (erf gelu)
    out   = h @ w2.T + b2
    y[b]  = x[b] + out.T.reshape(C, H, W)

Strategy (data-parallel over batch, 2 items per core, 8 cores):
  Everything stays in the transposed layout [C-on-partitions, N-free].
  All heavy matmuls run in fp8 DoubleRow perf mode (2 k-tiles folded per
  instruction, 0.5 cycles per output row):
    - qT   = proj_w @ X + pb      (bf16 matmul; DVE adds bias, emits e4m3)
    - S    = qT.T @ qT            (e4m3 x e4m3 DoubleRow)
    - E    = exp(S/16 - 4.5)      (ACT, emits e5m2; e5m2's wide exponent
                                   absorbs the full score range with a single
                                   global shift that cancels in softmax)
    - Z    = ones.T @ E           (e4m3 ones x e5m2 E DoubleRow; E symmetric
                                   so column sums equal row sums)
    - agg  = XT.T @ E             (e4m3 XT x e5m2 E DoubleRow), scaled by
                                   broadcast(1/Z) (gpsimd partition_broadcast)
    - mlp  = two more e4m3 DoubleRow layers; gelu fused on ACT.
  XT ships pre-transposed from the host (layout prep, zero flops), as do the
  fp8/bf16 weight casts; all big tensors ship p-major packed so each DMA is
  128 descriptors of >=2KB.  The two items are software-pipelined phase by
  phase so ACT (the bottleneck: 16 exps + 4 gelus) never starves, and ACT
  switches tables (exp<->gelu) only twice per launch.
"""

import os
import sys

import numpy as np

for _p in ("/opt/trn_rl_repo", "/root/.axon_site/_ro/trn_rl_repo"):
    if os.path.isdir(_p) and _p not in sys.path:
        sys.path.insert(0, _p)

import ml_dtypes
import concourse.bass as bass
import concourse.bacc as bacc
import concourse.mybir as mybir
from concourse import tile
from concourse.alu_op_type import AluOpType
from concourse.bass_utils import run_bass_kernel_spmd

F32 = mybir.dt.float32
BF16 = mybir.dt.bfloat16
E4 = mybir.dt.float8e4
E5 = mybir.dt.float8e5
AFT = mybir.ActivationFunctionType
DR = mybir.MatmulPerfMode.DoubleRow

NP_E4 = ml_dtypes.float8_e4m3
NP_E5 = ml_dtypes.float8_e5m2
NP_BF = ml_dtypes.bfloat16

C = 256          # channels
N = 1024         # nodes = H*W
CT = C // 128    # channel partition-tiles (2)
NT = N // 128    # node partition-tiles (8)
NF = N // 512    # node free-chunks of 512 (2)
N_CORES = 8
ITEMS = 2        # batch items per core (B=16 / 8 cores)
EXP_BIAS = -4.5  # global exp shift; cancels in softmax, keeps E in e5m2 range


def ts(i, size):
    return slice(i * size, (i + 1) * size)


def build_nc():
    nc = bacc.Bacc(None, target_bir_lowering=False)

    xbf_d = nc.dram_tensor("xbf", [ITEMS, 128, CT * N], BF16, kind="ExternalInput")
    xt8_d = nc.dram_tensor("xt8", [ITEMS, 128, NT * C], E4, kind="ExternalInput")
    pwT_d = nc.dram_tensor("pwT", [C, C], BF16, kind="ExternalInput")
    w1T_d = nc.dram_tensor("w1T", [C, C], E4, kind="ExternalInput")
    w2T_d = nc.dram_tensor("w2T", [C, C], E4, kind="ExternalInput")
    onesdr_d = nc.dram_tensor("onesdr", [128, 2, 16], E4, kind="ExternalInput")
    pb_d = nc.dram_tensor("pb", [128, CT], F32, kind="ExternalInput")
    b1_d = nc.dram_tensor("b1", [128, CT], F32, kind="ExternalInput")
    b2_d = nc.dram_tensor("b2", [128, CT], F32, kind="ExternalInput")
    eb_d = nc.dram_tensor("eb", [128, 1], F32, kind="ExternalInput")
    y_d = nc.dram_tensor("y", [ITEMS, 128, CT * N], BF16, kind="ExternalOutput")

    with tile.TileContext(nc) as tc:
        with (
            tc.tile_pool(name="const", bufs=1) as constp,
            tc.tile_pool(name="xin", bufs=2) as xp,
            tc.tile_pool(name="xtp", bufs=2) as xtp,
            tc.tile_pool(name="qt", bufs=2) as qp,
            tc.tile_pool(name="ebig", bufs=2) as ep,
            tc.tile_pool(name="aggp", bufs=2) as aggp,
            tc.tile_pool(name="htp", bufs=2) as hp,
            tc.tile_pool(name="yp", bufs=2) as yp,
            tc.tile_pool(name="statp", bufs=2) as statp,
            tc.tile_pool(name="psa", bufs=2, space=bass.MemorySpace.PSUM) as psa,
            tc.tile_pool(name="psb", bufs=4, space=bass.MemorySpace.PSUM) as psb,
        ):
            # ---- constants ----
            pwT = constp.tile([128, CT, C], BF16)
            w1T = constp.tile([128, CT, C], E4)
            w2T = constp.tile([128, CT, C], E4)
            onesdr = constp.tile([128, 2, 16], E4)
            pb = constp.tile([128, CT], F32)
            b1 = constp.tile([128, CT], F32)
            b2 = constp.tile([128, CT], F32)
            eb = constp.tile([128, 1], F32)

            wtile = constp.tile([128, 512], BF16)
            nc.gpsimd.memset(wtile[:], 1.0)
            wps = psb.tile([1, 512], F32, tag="mmb", name="warm")
            for _ in range(7):
                nc.tensor.matmul(wps[:], wtile[:, 0:1], wtile[:],
                                 start=True, stop=True)

            # per-item tiles, allocated up front so emission can interleave
            Xbf = [xp.tile([128, CT, N], BF16, tag="X", name=f"X{i}")
                   for i in range(ITEMS)]
            XT8 = [xtp.tile([128, NT, C], E4, tag="XT", name=f"XT{i}")
                   for i in range(ITEMS)]
            qT8 = [qp.tile([128, CT, N], E4, tag="qT", name=f"qT{i}")
                   for i in range(ITEMS)]
            E5t = [ep.tile([128, NT, N], E5, tag="E", name=f"E{i}")
                   for i in range(ITEMS)]
            rrow = [statp.tile([1, N], F32, tag="rrow", name=f"rr{i}")
                    for i in range(ITEMS)]
            Rbs = [statp.tile([128, N], F32, tag="rbs", name=f"Rbs{i}")
                   for i in range(ITEMS)]
            agg8 = [aggp.tile([128, CT, N], E4, tag="aggT", name=f"agg{i}")
                    for i in range(ITEMS)]
            hT8 = [hp.tile([128, CT, N], E4, tag="hT", name=f"hT{i}")
                   for i in range(ITEMS)]
            Y = [yp.tile([128, CT, N], BF16, tag="Y", name=f"Y{i}")
                 for i in range(ITEMS)]


            # ---- DMA: SP queue gets the critical path (pwT, item0) ----
            # SP/HWDGE queue: the startup-critical loads, in dependency order
            nc.sync.dma_start(pwT[:], pwT_d.ap().rearrange("(t p) m -> p t m", p=128))
            nc.sync.dma_start(Xbf[0][:], xbf_d.ap()[0])
            nc.sync.dma_start(XT8[0][:], xt8_d.ap()[0])
            nc.sync.dma_start(onesdr[:], onesdr_d.ap())
            # SWDGE (gpsimd) queue: small constants + item1 + pass-B weights
            nc.gpsimd.dma_start(pb[:], pb_d.ap())
            nc.gpsimd.dma_start(eb[:], eb_d.ap())
            nc.gpsimd.dma_start(Xbf[1][:], xbf_d.ap()[1])
            nc.gpsimd.dma_start(XT8[1][:], xt8_d.ap()[1])
            nc.gpsimd.dma_start(
                w1T[:], w1T_d.ap().rearrange("(t p) m -> p t m", p=128))
            nc.gpsimd.dma_start(
                w2T[:], w2T_d.ap().rearrange("(t p) m -> p t m", p=128))
            nc.gpsimd.dma_start(b1[:], b1_d.ap())
            nc.gpsimd.dma_start(b2[:], b2_d.ap())

            def proj(it):
                """qT = proj_w @ X + pb, emitted nf-major so S can start
                after the nf=0 bias chunks."""
                for nf in range(NF):
                    pss = [psb.tile([128, 512], F32, tag="mmb",
                                    name=f"q{it}_{mt}_{nf}") for mt in range(CT)]
                    for mt in range(CT):
                        for kt in range(CT):
                            nc.tensor.matmul(
                                pss[mt][:],
                                pwT[:, kt, ts(mt, 128)],
                                Xbf[it][:, kt, ts(nf, 512)],
                                start=(kt == 0),
                                stop=(kt == CT - 1),
                            )
                    for mt in range(CT):
                        if it == 0 and nf == 1:
                            nc.scalar.activation(
                                qT8[it][:, mt, ts(nf, 512)],
                                pss[mt][:],
                                AFT.Identity, bias=pb[:, mt : mt + 1],
                            )
                        else:
                            nc.vector.tensor_scalar(
                                qT8[it][:, mt, ts(nf, 512)],
                                pss[mt][:],
                                pb[:, mt : mt + 1], None, AluOpType.add,
                            )

            def s_exp(it, nt):
                """one row-block: S matmuls + exp; Z chain links after odd nt."""
                ps = psa.tile([128, N], F32, tag="mm", name=f"s{it}_{nt}")
                for mf in range(NF):
                    nc.tensor.matmul(
                        ps[:, ts(mf, 512)],
                        qT8[it][:, :, ts(nt, 128)],
                        qT8[it][:, :, ts(mf, 512)],
                        start=True, stop=True, perf_mode=DR,
                    )
                nc.scalar.activation(
                    E5t[it][:, nt, :], ps[:], AFT.Exp,
                    bias=eb[:, 0:1], scale=0.0625,
                )

            zts = {}

            def zchain(it, tps):
                """Z column-sum chain links (accumulating ones-matmuls)."""
                if it not in zts:
                    zts[it] = [psb.tile([1, 512], F32, tag="mmb",
                                        name=f"z{it}_{m}") for m in range(NF)]
                for tp in tps:
                    for mf in range(NF):
                        nc.tensor.matmul(
                            zts[it][mf][:],
                            onesdr[:, :, 0:1],
                            E5t[it][:, 2 * tp : 2 * tp + 2, ts(mf, 512)],
                            start=(tp == 0), stop=(tp == NT // 2 - 1),
                            perf_mode=DR, skip_group_check=True,
                        )

            def recip_bcast(it):
                for mf in range(NF):
                    nc.vector.reciprocal(
                        rrow[it][0:1, ts(mf, 512)], zts[it][mf][:])
                    nc.gpsimd.partition_broadcast(
                        Rbs[it][:, ts(mf, 512)], rrow[it][0:1, ts(mf, 512)])

            def agg(it, big=False):
                """agg matmuls nf-major + DVE normalize to e4m3.  big=True
                uses the psa pool (free after the last exp) so the final
                item's agg is not gated by the psb mlp rotation."""
                if big:
                    pss = [psa.tile([128, N], F32, tag="mm", name=f"g{it}_{ct}")
                           for ct in range(CT)]
                    get = lambda ct, nf: pss[ct][:, ts(nf, 512)]
                else:
                    tiles = {}
                    def get(ct, nf):
                        if (ct, nf) not in tiles:
                            tiles[(ct, nf)] = psb.tile(
                                [128, 512], F32, tag="mmb", name=f"g{it}_{ct}_{nf}")
                        return tiles[(ct, nf)][:]
                for nf in range(NF):
                    for ct in range(CT):
                        for tp in range(NT // 2):
                            nc.tensor.matmul(
                                get(ct, nf),
                                XT8[it][:, 2 * tp : 2 * tp + 2, ts(ct, 128)],
                                E5t[it][:, 2 * tp : 2 * tp + 2, ts(nf, 512)],
                                start=(tp == 0), stop=(tp == NT // 2 - 1),
                                perf_mode=DR,
                            )
                    for ct in range(CT):
                        with tc.high_priority(offset=100 if big else None):
                            nc.vector.tensor_tensor(
                                agg8[it][:, ct, ts(nf, 512)],
                                get(ct, nf),
                                Rbs[it][:, ts(nf, 512)],
                                AluOpType.mult,
                            )

            def mlp1(it):
                for mt in range(CT):
                    for nf in range(NF):
                        ps = psb.tile([128, 512], F32, tag="mmb",
                                      name=f"h{it}_{mt}_{nf}")
                        nc.tensor.matmul(
                            ps[:],
                            w1T[:, :, ts(mt, 128)],
                            agg8[it][:, :, ts(nf, 512)],
                            start=True, stop=True, perf_mode=DR,
                        )
                        nc.scalar.activation(
                            hT8[it][:, mt, ts(nf, 512)],
                            ps[:], AFT.Gelu,
                            bias=b1[:, mt : mt + 1],
                        )

            def mlp2(it):
                for mt in range(CT):
                    for nf in range(NF):
                        ps = psb.tile([128, 512], F32, tag="mmb",
                                      name=f"o{it}_{mt}_{nf}")
                        nc.tensor.matmul(
                            ps[:],
                            w2T[:, :, ts(mt, 128)],
                            hT8[it][:, :, ts(nf, 512)],
                            start=True, stop=True, perf_mode=DR,
                        )
                        nc.vector.scalar_tensor_tensor(
                            Y[it][:, mt, ts(nf, 512)],
                            ps[:],
                            b2[:, mt : mt + 1],
                            Xbf[it][:, mt, ts(nf, 512)],
                            AluOpType.add, AluOpType.add,
                        )
                        nc.sync.dma_start(
                            y_d.ap()[it][:, ts(mt * NF + nf, 512)],
                            Y[it][:, mt, ts(nf, 512)],
                        )

            # ---- software-pipelined emission ----
            proj(0)
            for nt in range(5):
                s_exp(0, nt)
            proj(1)                      # overlaps exp0 on ACT
            s_exp(0, 5)
            zchain(0, range(3))
            s_exp(0, 6)
            s_exp(0, 7)
            zchain(0, [3])
            recip_bcast(0)
            s_exp(1, 0)
            s_exp(1, 1)
            agg(0)                       # E0 complete; runs during exp1
            for nt in range(2, 6):
                s_exp(1, nt)
            zchain(1, range(3))
            s_exp(1, 6)
            s_exp(1, 7)
            zchain(1, [3])
            with tc.high_priority():
                recip_bcast(1)
            agg(1, big=True)
            mlp1(0)
            mlp2(0)
            mlp1(1)
            mlp2(1)

    nc.compile()
    return nc


_NC_CACHE = {}


def _get_nc():
    if "nc" not in _NC_CACHE:
        _NC_CACHE["nc"] = build_nc()
    return _NC_CACHE["nc"]


def make_in_maps(x, proj_w, proj_b, w1, b1, w2, b2):
    B = x.shape[0]
    xs = x.reshape(B, C, N)
    # p-major packing: [B, 128, CT*N], row p holds channels {p, 128+p}
    xbf = np.ascontiguousarray(
        xs.reshape(B, CT, 128, N).transpose(0, 2, 1, 3).reshape(B, 128, CT * N)
    ).astype(NP_BF)
    # XT p-major: [B, 128, NT*C], row p holds nodes {p, 128+p, ..., 896+p}
    xt8 = np.ascontiguousarray(
        xs.transpose(0, 2, 1).reshape(B, NT, 128, C).transpose(0, 2, 1, 3)
        .reshape(B, 128, NT * C)
    ).astype(NP_E4)
    shared = {
        "pwT": np.ascontiguousarray(proj_w.T).astype(NP_BF),
        "w1T": np.ascontiguousarray(w1.T).astype(NP_E4),
        "w2T": np.ascontiguousarray(w2.T).astype(NP_E4),
        "onesdr": np.ones((128, 2, 16), dtype=NP_E4),
        "pb": np.ascontiguousarray(proj_b.reshape(CT, 128).T.astype(np.float32)),
        "b1": np.ascontiguousarray(b1.reshape(CT, 128).T.astype(np.float32)),
        "b2": np.ascontiguousarray(b2.reshape(CT, 128).T.astype(np.float32)),
        "eb": np.full((128, 1), EXP_BIAS, dtype=np.float32),
    }
    in_maps = []
    for c in range(N_CORES):
        m = dict(shared)
        m["xbf"] = np.ascontiguousarray(xbf[c * ITEMS : (c + 1) * ITEMS])
        m["xt8"] = np.ascontiguousarray(xt8[c * ITEMS : (c + 1) * ITEMS])
        in_maps.append(m)
    return in_maps


def kernel(x, proj_w, proj_b, w1, b1, w2, b2, _trace=False, **trace_kw):
    nc = _get_nc()
    in_maps = make_in_maps(x, proj_w, proj_b, w1, b1, w2, b2)
    res = run_bass_kernel_spmd(
        nc, in_maps, list(range(N_CORES)), trace=_trace, **trace_kw
    )
    B, _, H, W = x.shape
    # unpack p-major bf16 [128, CT*N] -> [C, N] f32
    outs = [
        np.asarray(r["y"]).astype(np.float32)
        .reshape(ITEMS, 128, CT, N).transpose(0, 2, 1, 3).reshape(ITEMS, C, N)
        for r in res.results
    ]
    y = np.concatenate(outs, axis=0).reshape(B, C, H, W).astype(np.float32)
    if _trace:
        kernel.last_result = res
    return y


# revision 23
# speedup vs baseline: 1.1485x; 1.0597x over previous
"""Trainium2 Bass kernel for a dense graph-transformer block (fp8 DoubleRow).

Reference computation (per batch item b, with C=256, N=H*W=1024):
    nodes = x[b].reshape(C, N).T                      # [N, C]
    q     = nodes @ proj_w.T + proj_b                 # [N, C]
    S     = (q @ q.T) / sqrt(C)                       # [N, N]  (symmetric!)
    A     = softmax(S, axis=-1)
    agg   = A @ nodes                                 # [N, C]
    h     = gelu(agg @ w1.T + b1)  (erf gelu)
    out   = h @ w2.T + b2
    y[b]  = x[b] + out.T.reshape(C, H, W)

Strategy (data-parallel over batch, 2 items per core, 8 cores):
  Everything stays in the transposed layout [C-on-partitions, N-free].
  All heavy matmuls run in fp8 DoubleRow perf mode (two 128-deep k-tiles
  folded per instruction, 0.5 cycles per output row):
    - qT   = proj_w @ X + pb      (e4m3 DoubleRow; DVE/ACT add the bias and
                                   emit e4m3)
    - S    = qT.T @ qT            (e4m3 x e4m3 DoubleRow)
    - E    = exp(S/16 - 4.5)      (ACT, emits e5m2; e5m2's wide exponent
                                   absorbs the full score range with a single
                                   global shift that cancels in softmax)
    - Z    = ones.T @ E           (e4m3 ones x e5m2 E DoubleRow; E symmetric
                                   so column sums equal row sums)
    - agg  = XT.T @ E             (e4m3 XT x e5m2 E DoubleRow), scaled by
                                   broadcast(1/Z) (gpsimd partition_broadcast)
    - mlp  = two more e4m3 DoubleRow layers; gelu fused on ACT.
  XT ships pre-transposed from the host (layout prep, zero flops), as do the
  fp8/bf16 casts; big tensors ship p-major packed so each DMA is 128
  descriptors of >=1KB.  The ACT engine is the bottleneck (16 exps + 8 gelu
  chunks); the two items are software-pipelined so ACT never starves, act
  tables switch (exp<->gelu) only twice, junk matmuls warm the PE clock
  before the first real matmul, and the critical startup DMAs (fp8 x, fp8
  proj weights) are small and first in queue.
"""

import os
import sys

import numpy as np

for _p in ("/opt/trn_rl_repo", "/root/.axon_site/_ro/trn_rl_repo"):
    if os.path.isdir(_p) and _p not in sys.path:
        sys.path.insert(0, _p)

import ml_dtypes
import concourse.bass as bass
import concourse.bacc as bacc
import concourse.mybir as mybir
from concourse import tile
from concourse.alu_op_type import AluOpType
from concourse.bass_utils import run_bass_kernel_spmd

F32 = mybir.dt.float32
BF16 = mybir.dt.bfloat16
E4 = mybir.dt.float8e4
E5 = mybir.dt.float8e5
AFT = mybir.ActivationFunctionType
DR = mybir.MatmulPerfMode.DoubleRow

NP_E4 = ml_dtypes.float8_e4m3
NP_E5 = ml_dtypes.float8_e5m2
NP_BF = ml_dtypes.bfloat16

C = 256          # channels
N = 1024         # nodes = H*W
CT = C // 128    # channel partition-tiles (2)
NT = N // 128    # node partition-tiles (8)
NF = N // 512    # node free-chunks of 512 (2)
N_CORES = 8
ITEMS = 2        # batch items per core (B=16 / 8 cores)
EXP_BIAS = -4.5  # global exp shift; cancels in softmax, keeps E in e5m2 range


def ts(i, size):
    return slice(i * size, (i + 1) * size)


def build_nc():
    nc = bacc.Bacc(None, target_bir_lowering=False)

    xq8_d = nc.dram_tensor("xq8", [ITEMS, 128, CT * N], E4, kind="ExternalInput")
    xbf_d = nc.dram_tensor("xbf", [ITEMS, 128, CT * N], BF16, kind="ExternalInput")
    xt8_d = nc.dram_tensor("xt8", [ITEMS, 128, NT * C], E4, kind="ExternalInput")
    pw8_d = nc.dram_tensor("pw8", [C, C], E4, kind="ExternalInput")
    w1T_d = nc.dram_tensor("w1T", [C, C], E4, kind="ExternalInput")
    w2T_d = nc.dram_tensor("w2T", [C, C], E4, kind="ExternalInput")
    onesdr_d = nc.dram_tensor("onesdr", [128, 2, 16], E4, kind="ExternalInput")
    pb_d = nc.dram_tensor("pb", [128, CT], F32, kind="ExternalInput")
    b1_d = nc.dram_tensor("b1", [128, CT], F32, kind="ExternalInput")
    b2_d = nc.dram_tensor("b2", [128, CT], F32, kind="ExternalInput")
    eb_d = nc.dram_tensor("eb", [128, 1], F32, kind="ExternalInput")
    y_d = nc.dram_tensor("y", [ITEMS, 128, CT * N], BF16, kind="ExternalOutput")

    with tile.TileContext(nc) as tc:
        with (
            tc.tile_pool(name="const", bufs=1) as constp,
            tc.tile_pool(name="xq", bufs=2) as xqp,
            tc.tile_pool(name="xin", bufs=2) as xp,
            tc.tile_pool(name="xtp", bufs=2) as xtp,
            tc.tile_pool(name="qt", bufs=2) as qp,
            tc.tile_pool(name="ebig", bufs=2) as ep,
            tc.tile_pool(name="aggp", bufs=2) as aggp,
            tc.tile_pool(name="htp", bufs=2) as hp,
            tc.tile_pool(name="yp", bufs=2) as yp,
            tc.tile_pool(name="statp", bufs=2) as statp,
            tc.tile_pool(name="psa", bufs=2, space=bass.MemorySpace.PSUM) as psa,
            tc.tile_pool(name="psb", bufs=4, space=bass.MemorySpace.PSUM) as psb,
        ):
            # ---- constants ----
            pw8 = constp.tile([128, CT, C], E4)
            w1T = constp.tile([128, CT, C], E4)
            w2T = constp.tile([128, CT, C], E4)
            onesdr = constp.tile([128, 2, 16], E4)
            pb = constp.tile([128, CT], F32)
            b1 = constp.tile([128, CT], F32)
            b2 = constp.tile([128, CT], F32)
            eb = constp.tile([128, 1], F32)

            # warm the PE clock: junk matmuls until the first real ones land
            wtile = constp.tile([128, 512], BF16)
            nc.gpsimd.memset(wtile[:], 1.0)
            wps = psb.tile([1, 512], F32, tag="mmb", name="warm")
            for _ in range(7):
                nc.tensor.matmul(wps[:], wtile[:, 0:1], wtile[:],
                                 start=True, stop=True)

            # per-item tiles, allocated up front so emission can interleave
            Xq8 = [xqp.tile([128, CT, N], E4, tag="Xq", name=f"Xq{i}")
                   for i in range(ITEMS)]
            Xbf = [xp.tile([128, CT, N], BF16, tag="X", name=f"X{i}")
                   for i in range(ITEMS)]
            XT8 = [xtp.tile([128, NT, C], E4, tag="XT", name=f"XT{i}")
                   for i in range(ITEMS)]
            qT8 = [qp.tile([128, CT, N], E4, tag="qT", name=f"qT{i}")
                   for i in range(ITEMS)]
            E5t = [ep.tile([128, NT, N], E5, tag="E", name=f"E{i}")
                   for i in range(ITEMS)]
            rrow = [statp.tile([1, N], F32, tag="rrow", name=f"rr{i}")
                    for i in range(ITEMS)]
            Rbs = [statp.tile([128, N], F32, tag="rbs", name=f"Rbs{i}")
                   for i in range(ITEMS)]
            agg8 = [aggp.tile([128, CT, N], E4, tag="aggT", name=f"agg{i}")
                    for i in range(ITEMS)]
            hT8 = [hp.tile([128, CT, N], E4, tag="hT", name=f"hT{i}")
                   for i in range(ITEMS)]
            Y = [yp.tile([128, CT, N], BF16, tag="Y", name=f"Y{i}")
                 for i in range(ITEMS)]

            # ---- DMA: SP/HWDGE queue carries the startup-critical loads ----
            nc.sync.dma_start(pw8[:], pw8_d.ap().rearrange("(t p) m -> p t m", p=128))
            nc.sync.dma_start(Xq8[0][:], xq8_d.ap()[0])
            nc.sync.dma_start(Xq8[1][:], xq8_d.ap()[1])
            nc.sync.dma_start(XT8[0][:], xt8_d.ap()[0])
            nc.sync.dma_start(onesdr[:], onesdr_d.ap())
            # SWDGE (gpsimd) queue: small consts + lazy loads for later phases
            nc.gpsimd.dma_start(pb[:], pb_d.ap())
            nc.gpsimd.dma_start(eb[:], eb_d.ap())
            nc.gpsimd.dma_start(Xbf[0][:], xbf_d.ap()[0])
            nc.gpsimd.dma_start(XT8[1][:], xt8_d.ap()[1])
            nc.gpsimd.dma_start(Xbf[1][:], xbf_d.ap()[1])
            nc.gpsimd.dma_start(
                w1T[:], w1T_d.ap().rearrange("(t p) m -> p t m", p=128))
            nc.gpsimd.dma_start(
                w2T[:], w2T_d.ap().rearrange("(t p) m -> p t m", p=128))
            nc.gpsimd.dma_start(b1[:], b1_d.ap())
            nc.gpsimd.dma_start(b2[:], b2_d.ap())

            def proj(it):
                """qT = proj_w @ X + pb (e4m3 DoubleRow), nf-major; item0's
                nf=1 bias chunks go to the (startup-idle) ACT engine."""
                for nf in range(NF):
                    pss = [psb.tile([128, 512], F32, tag="mmb",
                                    name=f"q{it}_{mt}_{nf}") for mt in range(CT)]
                    for mt in range(CT):
                        nc.tensor.matmul(
                            pss[mt][:],
                            pw8[:, :, ts(mt, 128)],
                            Xq8[it][:, :, ts(nf, 512)],
                            start=True, stop=True, perf_mode=DR,
                        )
                    for mt in range(CT):
                        if it == 0 and nf == 1:
                            nc.scalar.activation(
                                qT8[it][:, mt, ts(nf, 512)],
                                pss[mt][:],
                                AFT.Identity, bias=pb[:, mt : mt + 1],
                            )
                        else:
                            nc.vector.tensor_scalar(
                                qT8[it][:, mt, ts(nf, 512)],
                                pss[mt][:],
                                pb[:, mt : mt + 1], None, AluOpType.add,
                            )

            def s_exp(it, nt):
                """one row-block: S DoubleRow matmuls + exp -> e5m2."""
                ps = psa.tile([128, N], F32, tag="mm", name=f"s{it}_{nt}")
                for mf in range(NF):
                    nc.tensor.matmul(
                        ps[:, ts(mf, 512)],
                        qT8[it][:, :, ts(nt, 128)],
                        qT8[it][:, :, ts(mf, 512)],
                        start=True, stop=True, perf_mode=DR,
                    )
                nc.scalar.activation(
                    E5t[it][:, nt, :], ps[:], AFT.Exp,
                    bias=eb[:, 0:1], scale=0.0625,
                )

            zts = {}

            def zchain(it, tps):
                """Z column-sum chain links (accumulating ones-matmuls)."""
                if it not in zts:
                    zts[it] = [psb.tile([1, 512], F32, tag="mmb",
                                        name=f"z{it}_{m}") for m in range(NF)]
                for tp in tps:
                    for mf in range(NF):
                        nc.tensor.matmul(
                            zts[it][mf][:],
                            onesdr[:, :, 0:1],
                            E5t[it][:, 2 * tp : 2 * tp + 2, ts(mf, 512)],
                            start=(tp == 0), stop=(tp == NT // 2 - 1),
                            perf_mode=DR, skip_group_check=True,
                        )

            def recip_bcast(it):
                for mf in range(NF):
                    nc.vector.reciprocal(
                        rrow[it][0:1, ts(mf, 512)], zts[it][mf][:])
                    nc.gpsimd.partition_broadcast(
                        Rbs[it][:, ts(mf, 512)], rrow[it][0:1, ts(mf, 512)])

            def agg(it, big=False):
                """agg DoubleRow matmuls nf-major + DVE normalize to e4m3.
                big=True uses the psa pool (free after the last exp) so the
                final item's agg is not gated by the psb mlp rotation."""
                if big:
                    pss = [psa.tile([128, N], F32, tag="mm", name=f"g{it}_{ct}")
                           for ct in range(CT)]
                    get = lambda ct, nf: pss[ct][:, ts(nf, 512)]
                else:
                    tiles = {}
                    def get(ct, nf):
                        if (ct, nf) not in tiles:
                            tiles[(ct, nf)] = psb.tile(
                                [128, 512], F32, tag="mmb", name=f"g{it}_{ct}_{nf}")
                        return tiles[(ct, nf)][:]
                for nf in range(NF):
                    for ct in range(CT):
                        for tp in range(NT // 2):
                            nc.tensor.matmul(
                                get(ct, nf),
                                XT8[it][:, 2 * tp : 2 * tp + 2, ts(ct, 128)],
                                E5t[it][:, 2 * tp : 2 * tp + 2, ts(nf, 512)],
                                start=(tp == 0), stop=(tp == NT // 2 - 1),
                                perf_mode=DR,
                            )
                    for ct in range(CT):
                        with tc.high_priority(offset=100 if big else None):
                            nc.vector.tensor_tensor(
                                agg8[it][:, ct, ts(nf, 512)],
                                get(ct, nf),
                                Rbs[it][:, ts(nf, 512)],
                                AluOpType.mult,
                            )

            def mlp1(it):
                for mt in range(CT):
                    for nf in range(NF):
                        ps = psb.tile([128, 512], F32, tag="mmb",
                                      name=f"h{it}_{mt}_{nf}")
                        nc.tensor.matmul(
                            ps[:],
                            w1T[:, :, ts(mt, 128)],
                            agg8[it][:, :, ts(nf, 512)],
                            start=True, stop=True, perf_mode=DR,
                        )
                        nc.scalar.activation(
                            hT8[it][:, mt, ts(nf, 512)],
                            ps[:], AFT.Gelu,
                            bias=b1[:, mt : mt + 1],
                        )

            def mlp2(it):
                for mt in range(CT):
                    for nf in range(NF):
                        ps = psb.tile([128, 512], F32, tag="mmb",
                                      name=f"o{it}_{mt}_{nf}")
                        nc.tensor.matmul(
                            ps[:],
                            w2T[:, :, ts(mt, 128)],
                            hT8[it][:, :, ts(nf, 512)],
                            start=True, stop=True, perf_mode=DR,
                        )
                        nc.vector.scalar_tensor_tensor(
                            Y[it][:, mt, ts(nf, 512)],
                            ps[:],
                            b2[:, mt : mt + 1],
                            Xbf[it][:, mt, ts(nf, 512)],
                            AluOpType.add, AluOpType.add,
                        )
                        nc.sync.dma_start(
                            y_d.ap()[it][:, ts(mt * NF + nf, 512)],
                            Y[it][:, mt, ts(nf, 512)],
                        )

            # ---- software-pipelined emission ----
            proj(0)
            for nt in range(5):
                s_exp(0, nt)
            proj(1)                      # overlaps exp0 on ACT
            s_exp(0, 5)
            zchain(0, range(3))
            s_exp(0, 6)
            s_exp(0, 7)
            zchain(0, [3])
            recip_bcast(0)
            s_exp(1, 0)
            s_exp(1, 1)
            agg(0)                       # E0 complete; runs during exp1
            for nt in range(2, 6):
                s_exp(1, nt)
            zchain(1, range(3))
            s_exp(1, 6)
            s_exp(1, 7)
            zchain(1, [3])
            with tc.high_priority():
                recip_bcast(1)
            agg(1, big=True)
            mlp1(0)
            mlp1(1)
            mlp2(0)
            mlp2(1)

    nc.compile()
    return nc


_NC_CACHE = {}


def _get_nc():
    if "nc" not in _NC_CACHE:
        _NC_CACHE["nc"] = build_nc()
    return _NC_CACHE["nc"]


def make_in_maps(x, proj_w, proj_b, w1, b1, w2, b2):
    B = x.shape[0]
    xs = x.reshape(B, C, N)
    # p-major packing: [B, 128, CT*N], row p holds channels {p, 128+p}
    xpk = np.ascontiguousarray(
        xs.reshape(B, CT, 128, N).transpose(0, 2, 1, 3).reshape(B, 128, CT * N)
    )
    xq8 = xpk.astype(NP_E4)
    xbf = xpk.astype(NP_BF)
    # XT p-major: [B, 128, NT*C], row p holds nodes {p, 128+p, ..., 896+p}
    xt8 = np.ascontiguousarray(
        xs.transpose(0, 2, 1).reshape(B, NT, 128, C).transpose(0, 2, 1, 3)
        .reshape(B, 128, NT * C)
    ).astype(NP_E4)
    shared = {
        "pw8": np.ascontiguousarray(proj_w.T).astype(NP_E4),
        "w1T": np.ascontiguousarray(w1.T).astype(NP_E4),
        "w2T": np.ascontiguousarray(w2.T).astype(NP_E4),
        "onesdr": np.ones((128, 2, 16), dtype=NP_E4),
        "pb": np.ascontiguousarray(proj_b.reshape(CT, 128).T.astype(np.float32)),
        "b1": np.ascontiguousarray(b1.reshape(CT, 128).T.astype(np.float32)),
        "b2": np.ascontiguousarray(b2.reshape(CT, 128).T.astype(np.float32)),
        "eb": np.full((128, 1), EXP_BIAS, dtype=np.float32),
    }
    in_maps = []
    for c in range(N_CORES):
        m = dict(shared)
        m["xq8"] = np.ascontiguousarray(xq8[c * ITEMS : (c + 1) * ITEMS])
        m["xbf"] = np.ascontiguousarray(xbf[c * ITEMS : (c + 1) * ITEMS])
        m["xt8"] = np.ascontiguousarray(xt8[c * ITEMS : (c + 1) * ITEMS])
        in_maps.append(m)
    return in_maps


def kernel(x, proj_w, proj_b, w1, b1, w2, b2, _trace=False, **trace_kw):
    nc = _get_nc()
    in_maps = make_in_maps(x, proj_w, proj_b, w1, b1, w2, b2)
    res = run_bass_kernel_spmd(
        nc, in_maps, list(range(N_CORES)), trace=_trace, **trace_kw
    )
    B, _, H, W = x.shape
    # unpack p-major bf16 [128, CT*N] -> [C, N] f32
    outs = [
        np.asarray(r["y"]).astype(np.float32)
        .reshape(ITEMS, 128, CT, N).transpose(0, 2, 1, 3).reshape(ITEMS, C, N)
        for r in res.results
    ]
    y = np.concatenate(outs, axis=0).reshape(B, C, H, W).astype(np.float32)
    if _trace:
        kernel.last_result = res
    return y
